# revision 3
# baseline (speedup 1.0000x reference)
"""MoSRAH router kernel for 8 trn2 NeuronCores (Bass/Tile).

Strategy (data/sequence parallel per the sharding hint):
  - Flatten tokens (B*N = 16384) and shard 2048 tokens per core.
  - x is fed to each core pre-transposed ([H, T]) so the contraction dim (H)
    lands on SBUF partitions for the matmul; W_r / expert_bias replicated.
  - Per core: logitsT = W_rT.T @ xT accumulated over 16 h-chunks in PSUM,
    bias added during the PSUM->SBUF copy (l is the partition dim there),
    PE-transposed back to [token, l] layout, then DVE max8/find_index8 for
    the top-8, match_replace + compare for the expert histogram, and a
    softmax over the 8 selected logits (the full softmax denominator
    cancels after renormalization, so only the selected logits matter).
  - Host: concat shards, sum the 8 per-core histograms and compute the
    two loss scalars (the all-reduce of a 64-vector).
"""

import sys

if "/opt/trn_rl_repo" not in sys.path:
    sys.path.insert(0, "/opt/trn_rl_repo")

from contextlib import ExitStack

import numpy as np

B, N, H, L, K = 4, 4096, 2048, 64, 8
N_CORES = 8
TC = (B * N) // N_CORES  # tokens per core = 2048
NTAU = 4                 # 512-token tiles per core
TT = 512
NG = 4                   # 128-token groups per tile
NGRP = NTAU * NG         # 16
HC = H // 128            # 16 h-chunks
BIG = 2.0e30
THRESH = 1.0e30

_CACHE = {}


def _build_nc():
    import concourse.bass as bass
    import concourse.tile as tile
    from concourse import bacc, mybir

    f32 = mybir.dt.float32
    u8 = mybir.dt.uint8
    u32 = mybir.dt.uint32

    nc = bacc.Bacc("TRN2", target_bir_lowering=False, debug=False,
                   num_devices=N_CORES)
    xT = nc.declare_dram_parameter("xT", [H, TC], f32, isOutput=False)
    wT = nc.declare_dram_parameter("wT", [H, L], f32, isOutput=False)
    bias = nc.declare_dram_parameter("bias", [L], f32, isOutput=False)
    mask = nc.declare_dram_parameter("mask", [TC], u8, isOutput=False)
    sel_o = nc.declare_dram_parameter("sel", [128, NGRP, K], u32, isOutput=True)
    prob_o = nc.declare_dram_parameter("probs", [128, NGRP, K], f32, isOutput=True)
    cnt_o = nc.declare_dram_parameter("cnt", [1, NG * L], f32, isOutput=True)

    xr = xT[:].rearrange("(c p) t -> p c t", p=128)  # h-chunk view of x^T

    with tile.TileContext(nc) as tc, ExitStack() as ctx:
        const = ctx.enter_context(tc.tile_pool(name="const", bufs=1))
        xp = ctx.enter_context(tc.tile_pool(name="x", bufs=2))
        wk = ctx.enter_context(tc.tile_pool(name="wk", bufs=2))
        ps_lt = ctx.enter_context(tc.tile_pool(name="ps_lt", bufs=2, space="PSUM"))
        ps_tp = ctx.enter_context(tc.tile_pool(name="ps_tp", bufs=2, space="PSUM"))
        ps_cnt = ctx.enter_context(tc.tile_pool(name="ps_cnt", bufs=1, space="PSUM"))

        # --- one-time constants -------------------------------------------
        w_sb = const.tile([128, HC, L], f32)
        nc.sync.dma_start(w_sb[:], wT[:].rearrange("(c p) l -> p c l", p=128))

        b_sb = const.tile([64, 1], f32)
        nc.sync.dma_start(b_sb[:], bias[:].rearrange("(a b) -> a b", b=1))

        m_u8 = const.tile([128, NTAU, NG], u8)
        nc.sync.dma_start(
            m_u8[:], mask[:].rearrange("(tau gl p) -> p tau gl", p=128, gl=NG)
        )
        m_f = const.tile([128, NTAU, NG], f32)
        nc.vector.tensor_copy(m_f[:], m_u8[:])

        ones64 = const.tile([64, 64], f32)
        nc.gpsimd.memset(ones64[:], 1.0)
        ident = const.tile([64, 64], f32)
        nc.gpsimd.affine_select(
            ident[:], ones64[:], pattern=[[-1, 64]],
            compare_op=mybir.AluOpType.is_equal, fill=0.0,
            base=0, channel_multiplier=1,
        )
        ones128 = const.tile([128, 1], f32)
        nc.gpsimd.memset(ones128[:], 1.0)

        acc = const.tile([128, NG, L], f32)
        nc.vector.memset(acc[:], 0.0)

        vals = const.tile([128, NGRP, K], f32)
        idx = const.tile([128, NGRP, K], u32)

        # --- main loop over 512-token tiles -------------------------------
        for tau in range(NTAU):
            x_sb = xp.tile([128, HC, TT], f32)
            for q in range(4):
                nc.sync.dma_start(
                    x_sb[:][:, 4 * q:4 * q + 4, :],
                    xr[:, 4 * q:4 * q + 4, tau * TT:(tau + 1) * TT],
                )

            lt = ps_lt.tile([64, TT], f32)
            for c in range(HC):
                nc.tensor.matmul(
                    lt[:], lhsT=w_sb[:][:, c, :], rhs=x_sb[:][:, c, :],
                    start=(c == 0), stop=(c == HC - 1),
                )

            # biased logits^T in SBUF (bias is per-partition here)
            yT = wk.tile([64, TT], f32, tag="yT")
            nc.vector.tensor_scalar_add(yT[:], lt[:], b_sb[:][:, 0:1])

            tp = ps_tp.tile([128, NG * L], f32)
            for gl in range(NG):
                nc.tensor.transpose(
                    tp[:][:, gl * L:(gl + 1) * L],
                    yT[:][:, gl * 128:(gl + 1) * 128],
                    ident[:],
                )

            y_sb = wk.tile([128, NG, L], f32, tag="ysb")
            nc.scalar.copy(y_sb[:].rearrange("p a b -> p (a b)"), tp[:])

            rep = wk.tile([128, NG, L], f32, tag="rep")
            for gl in range(NG):
                g = tau * NG + gl
                nc.vector.max(vals[:][:, g], y_sb[:][:, gl])
                nc.vector.max_index(idx[:][:, g], vals[:][:, g], y_sb[:][:, gl])
                nc.vector.match_replace(rep[:][:, gl], vals[:][:, g], y_sb[:][:, gl], BIG)

            # histogram: (rep >= THRESH) * active_mask, accumulated
            m_sl = m_f[:][:, tau, :]
            m_bc = bass.AP(m_sl.tensor, m_sl.offset, m_sl.ap + [[0, L]])
            ohm = wk.tile([128, NG, L], f32, tag="ohm")
            nc.vector.scalar_tensor_tensor(
                ohm[:], rep[:], THRESH, m_bc,
                op0=mybir.AluOpType.is_ge, op1=mybir.AluOpType.mult,
            )
            nc.vector.tensor_add(acc[:], acc[:], ohm[:])

        # --- expert counts: reduce over partitions via ones-matmul --------
        cnt_ps = ps_cnt.tile([1, NG * L], f32)
        nc.tensor.matmul(
            cnt_ps[:], lhsT=ones128[:],
            rhs=acc[:].rearrange("p a b -> p (a b)"),
            start=True, stop=True,
        )
        cnt_sb = const.tile([1, NG * L], f32)
        nc.vector.tensor_copy(cnt_sb[:], cnt_ps[:])
        nc.sync.dma_start(cnt_o[:], cnt_sb[:])

        # --- softmax over the 8 selected logits (batched) -----------------
        vmax = vals[:][:, :, 0:1].broadcast_to((128, NGRP, K))
        d = const.tile([128, NGRP, K], f32)
        nc.vector.tensor_sub(d[:], vals[:], vmax)
        e = const.tile([128, NGRP, K], f32)
        nc.scalar.activation(e[:], d[:], mybir.ActivationFunctionType.Exp)
        s = const.tile([128, NGRP], f32)
        nc.vector.reduce_sum(s[:], e[:], axis=mybir.AxisListType.X)
        r = const.tile([128, NGRP], f32)
        nc.vector.reciprocal(r[:], s[:])
        r_ap = r[:]
        r_bc = bass.AP(r_ap.tensor, r_ap.offset, r_ap.ap + [[0, K]])
        pr = const.tile([128, NGRP, K], f32)
        nc.vector.tensor_mul(pr[:], e[:], r_bc)

        nc.sync.dma_start(prob_o[:], pr[:])
        nc.sync.dma_start(sel_o[:], idx[:])

    nc.finalize()
    return nc


def _get_nc():
    if "nc" not in _CACHE:
        _CACHE["nc"] = _build_nc()
    return _CACHE["nc"]


def _make_in_maps(x, W_r, expert_bias, active_mask):
    xf = np.ascontiguousarray(np.asarray(x, dtype=np.float32).reshape(B * N, H))
    wT = np.ascontiguousarray(np.asarray(W_r, dtype=np.float32).T)
    bias = np.ascontiguousarray(np.asarray(expert_bias, dtype=np.float32))
    mf = np.asarray(active_mask).reshape(B * N).astype(np.uint8)
    in_maps = []
    for i in range(N_CORES):
        shard = xf[i * TC:(i + 1) * TC]
        in_maps.append(dict(
            xT=np.ascontiguousarray(shard.T),
            wT=wT,
            bias=bias,
            mask=np.ascontiguousarray(mf[i * TC:(i + 1) * TC]),
        ))
    return in_maps


def _assemble(results, active_mask):
    sels, ps = [], []
    cnt_total = np.zeros(L, np.float32)
    for r in results:
        s = r["sel"].reshape(128, NTAU, NG, K).transpose(1, 2, 0, 3).reshape(TC, K)
        p = r["probs"].reshape(128, NTAU, NG, K).transpose(1, 2, 0, 3).reshape(TC, K)
        sels.append(s.astype(np.int32))
        ps.append(p.astype(np.float32))
        cnt_total += r["cnt"].reshape(NG, L).sum(0, dtype=np.float32)
    sel_full = np.concatenate(sels, 0).reshape(B, N, K)
    probs_full = np.concatenate(ps, 0).reshape(B, N, K)
    am = np.asarray(active_mask).astype(np.float32)
    denom = np.float32(am.sum(dtype=np.float32) * K)
    freqs = (cnt_total / denom).astype(np.float32)
    inv_L = np.float32(1.0 / L)
    lbl = np.float32(L * np.sum((freqs - inv_L) ** 2, dtype=np.float32))
    vio = np.float32(L * np.max(freqs - inv_L))
    return sel_full, probs_full, lbl, vio


def run(x, W_r, expert_bias, active_mask, trace=False, **kw):
    from concourse.bass_utils import run_bass_kernel_spmd

    nc = _get_nc()
    in_maps = _make_in_maps(x, W_r, expert_bias, active_mask)
    res = run_bass_kernel_spmd(nc, in_maps, list(range(N_CORES)), trace=trace, **kw)
    return _assemble(res.results, active_mask), res


def kernel(x, W_r, expert_bias, active_mask):
    return run(x, W_r, expert_bias, active_mask)[0]


# revision 7
# speedup vs baseline: 1.3015x; 1.3015x over previous
"""MoSRAH router kernel for 8 trn2 NeuronCores (Bass/Tile).

Strategy (data/sequence parallel per the sharding hint):
  - Flatten tokens (B*N = 16384) and shard 2048 tokens per core.
  - x is fed to each core pre-transposed ([H, T]) so the contraction dim (H)
    lands on SBUF partitions for the matmul; W_r / expert_bias replicated.
  - Per core: logitsT = W_rT.T @ xT accumulated over 16 h-chunks in PSUM,
    bias added during the PSUM->SBUF copy (l is the partition dim there),
    PE-transposed back to [token, l] layout, then DVE max8/find_index8 for
    the top-8, match_replace + compare for the expert histogram, and a
    softmax over the 8 selected logits (the full softmax denominator
    cancels after renormalization, so only the selected logits matter).
  - Host: concat shards, sum the 8 per-core histograms and compute the
    two loss scalars (the all-reduce of a 64-vector).
"""

import sys

if "/opt/trn_rl_repo" not in sys.path:
    sys.path.insert(0, "/opt/trn_rl_repo")

from contextlib import ExitStack

import numpy as np

B, N, H, L, K = 4, 4096, 2048, 64, 8
N_CORES = 8
TC = (B * N) // N_CORES  # tokens per core = 2048
NTAU = 4                 # 512-token tiles per core
TT = 512
NG = 4                   # 128-token groups per tile
NGRP = NTAU * NG         # 16
HC = H // 128            # 16 h-chunks
BIG = 2.0e30
THRESH = 1.0e30

_CACHE = {}


def _build_nc():
    import concourse.bass as bass
    import concourse.tile as tile
    from concourse import bacc, mybir

    f32 = mybir.dt.float32
    f32r = mybir.dt.float32r  # full-rate fp32 matmul path (1 cyc/row at N>=256)
    u8 = mybir.dt.uint8
    u32 = mybir.dt.uint32

    nc = bacc.Bacc("TRN2", target_bir_lowering=False, debug=False,
                   num_devices=N_CORES)
    # x arrives host-permuted as [tau, partition(h%128), chunk(h//128), token]
    # so each DMA descriptor is one 16-32KB contiguous run per partition.
    xt = nc.declare_dram_parameter("xt", [NTAU, 128, HC, TT], f32r, isOutput=False)
    wT = nc.declare_dram_parameter("wT", [H, L], f32r, isOutput=False)
    bias = nc.declare_dram_parameter("bias", [L], f32, isOutput=False)
    mask = nc.declare_dram_parameter("mask", [TC], u8, isOutput=False)
    sel_o = nc.declare_dram_parameter("sel", [128, NGRP, K], u32, isOutput=True)
    prob_o = nc.declare_dram_parameter("probs", [128, NGRP, K], f32, isOutput=True)
    cnt_o = nc.declare_dram_parameter("cnt", [1, NG * L], f32, isOutput=True)

    with tile.TileContext(nc) as tc, ExitStack() as ctx:
        const = ctx.enter_context(tc.tile_pool(name="const", bufs=1))
        xp = ctx.enter_context(tc.tile_pool(name="x", bufs=2))
        wk = ctx.enter_context(tc.tile_pool(name="wk", bufs=2))
        ps_lt = ctx.enter_context(tc.tile_pool(name="ps_lt", bufs=2, space="PSUM"))
        ps_tp = ctx.enter_context(tc.tile_pool(name="ps_tp", bufs=2, space="PSUM"))
        ps_cnt = ctx.enter_context(tc.tile_pool(name="ps_cnt", bufs=1, space="PSUM"))

        # --- one-time constants -------------------------------------------
        w_sb = const.tile([128, HC, L], f32r)
        nc.sync.dma_start(w_sb[:], wT[:].rearrange("(c p) l -> p c l", p=128))

        b_sb = const.tile([64, 1], f32)
        nc.sync.dma_start(b_sb[:], bias[:].rearrange("(a b) -> a b", b=1))

        m_u8 = const.tile([128, NTAU, NG], u8)
        nc.sync.dma_start(
            m_u8[:], mask[:].rearrange("(tau gl p) -> p tau gl", p=128, gl=NG)
        )
        m_f = const.tile([128, NTAU, NG], f32)
        nc.vector.tensor_copy(m_f[:], m_u8[:])

        ones64 = const.tile([64, 64], f32)
        nc.gpsimd.memset(ones64[:], 1.0)
        ident = const.tile([64, 64], f32)
        nc.gpsimd.affine_select(
            ident[:], ones64[:], pattern=[[-1, 64]],
            compare_op=mybir.AluOpType.is_equal, fill=0.0,
            base=0, channel_multiplier=1,
        )
        ones128 = const.tile([128, 1], f32)
        nc.gpsimd.memset(ones128[:], 1.0)

        acc = const.tile([128, NG, L], f32)
        nc.vector.memset(acc[:], 0.0)

        vals = const.tile([128, NGRP, K], f32)
        idx = const.tile([128, NGRP, K], u32)

        # --- main loop over 512-token tiles -------------------------------
        for tau in range(NTAU):
            x_sb = xp.tile([128, HC, TT], f32r)
            for q in range(2):
                nc.sync.dma_start(
                    x_sb[:][:, 8 * q:8 * q + 8, :],
                    xt[:][tau, :, 8 * q:8 * q + 8, :],
                )

            lt = ps_lt.tile([64, TT], f32)
            for c in range(HC):
                nc.tensor.matmul(
                    lt[:], lhsT=w_sb[:][:, c, :], rhs=x_sb[:][:, c, :],
                    start=(c == 0), stop=(c == HC - 1),
                )

            # biased logits^T in SBUF (bias is per-partition here)
            yT = wk.tile([64, TT], f32, tag="yT")
            nc.vector.tensor_scalar_add(yT[:], lt[:], b_sb[:][:, 0:1])

            tp = ps_tp.tile([128, NG * L], f32)
            for gl in range(NG):
                nc.tensor.transpose(
                    tp[:][:, gl * L:(gl + 1) * L],
                    yT[:][:, gl * 128:(gl + 1) * 128],
                    ident[:],
                )

            y_sb = wk.tile([128, NG, L], f32, tag="ysb")
            nc.scalar.copy(y_sb[:].rearrange("p a b -> p (a b)"), tp[:])

            rep = wk.tile([128, NG, L], f32, tag="rep")
            for gl in range(NG):
                g = tau * NG + gl
                nc.vector.max(vals[:][:, g], y_sb[:][:, gl])
                nc.vector.max_index(idx[:][:, g], vals[:][:, g], y_sb[:][:, gl])
                nc.vector.match_replace(rep[:][:, gl], vals[:][:, g], y_sb[:][:, gl], BIG)

            # histogram: (rep >= THRESH) * active_mask, accumulated
            m_sl = m_f[:][:, tau, :]
            m_bc = bass.AP(m_sl.tensor, m_sl.offset, m_sl.ap + [[0, L]])
            ohm = wk.tile([128, NG, L], f32, tag="ohm")
            nc.vector.scalar_tensor_tensor(
                ohm[:], rep[:], THRESH, m_bc,
                op0=mybir.AluOpType.is_ge, op1=mybir.AluOpType.mult,
            )
            nc.vector.tensor_add(acc[:], acc[:], ohm[:])

        # --- expert counts: reduce over partitions via ones-matmul --------
        cnt_ps = ps_cnt.tile([1, NG * L], f32)
        nc.tensor.matmul(
            cnt_ps[:], lhsT=ones128[:],
            rhs=acc[:].rearrange("p a b -> p (a b)"),
            start=True, stop=True,
        )
        cnt_sb = const.tile([1, NG * L], f32)
        nc.vector.tensor_copy(cnt_sb[:], cnt_ps[:])
        nc.sync.dma_start(cnt_o[:], cnt_sb[:])

        # --- softmax over the 8 selected logits (batched) -----------------
        vmax = vals[:][:, :, 0:1].broadcast_to((128, NGRP, K))
        d = const.tile([128, NGRP, K], f32)
        nc.vector.tensor_sub(d[:], vals[:], vmax)
        e = const.tile([128, NGRP, K], f32)
        nc.scalar.activation(e[:], d[:], mybir.ActivationFunctionType.Exp)
        s = const.tile([128, NGRP], f32)
        nc.vector.reduce_sum(s[:], e[:], axis=mybir.AxisListType.X)
        r = const.tile([128, NGRP], f32)
        nc.vector.reciprocal(r[:], s[:])
        r_ap = r[:]
        r_bc = bass.AP(r_ap.tensor, r_ap.offset, r_ap.ap + [[0, K]])
        pr = const.tile([128, NGRP, K], f32)
        nc.vector.tensor_mul(pr[:], e[:], r_bc)

        nc.sync.dma_start(prob_o[:], pr[:])
        nc.sync.dma_start(sel_o[:], idx[:])

    nc.finalize()
    return nc


def _get_nc():
    if "nc" not in _CACHE:
        _CACHE["nc"] = _build_nc()
    return _CACHE["nc"]


def _make_in_maps(x, W_r, expert_bias, active_mask):
    xf = np.asarray(x, dtype=np.float32).reshape(B * N, H)
    wT = np.ascontiguousarray(np.asarray(W_r, dtype=np.float32).T)
    bias = np.ascontiguousarray(np.asarray(expert_bias, dtype=np.float32))
    mf = np.asarray(active_mask).reshape(B * N).astype(np.uint8)
    in_maps = []
    for i in range(N_CORES):
        shard = xf[i * TC:(i + 1) * TC]
        # [tau, t, c, p] -> [tau, p, c, t]
        xtile = np.ascontiguousarray(
            shard.reshape(NTAU, TT, HC, 128).transpose(0, 3, 2, 1)
        )
        in_maps.append(dict(
            xt=xtile,
            wT=wT,
            bias=bias,
            mask=np.ascontiguousarray(mf[i * TC:(i + 1) * TC]),
        ))
    return in_maps


def _assemble(results, active_mask):
    sels, ps = [], []
    cnt_total = np.zeros(L, np.float32)
    for r in results:
        s = r["sel"].reshape(128, NTAU, NG, K).transpose(1, 2, 0, 3).reshape(TC, K)
        p = r["probs"].reshape(128, NTAU, NG, K).transpose(1, 2, 0, 3).reshape(TC, K)
        sels.append(s.astype(np.int32))
        ps.append(p.astype(np.float32))
        cnt_total += r["cnt"].reshape(NG, L).sum(0, dtype=np.float32)
    sel_full = np.concatenate(sels, 0).reshape(B, N, K)
    probs_full = np.concatenate(ps, 0).reshape(B, N, K)
    am = np.asarray(active_mask).astype(np.float32)
    denom = np.float32(am.sum(dtype=np.float32) * K)
    freqs = (cnt_total / denom).astype(np.float32)
    inv_L = np.float32(1.0 / L)
    lbl = np.float32(L * np.sum((freqs - inv_L) ** 2, dtype=np.float32))
    vio = np.float32(L * np.max(freqs - inv_L))
    return sel_full, probs_full, lbl, vio


def run(x, W_r, expert_bias, active_mask, trace=False, **kw):
    from concourse.bass_utils import run_bass_kernel_spmd

    nc = _get_nc()
    in_maps = _make_in_maps(x, W_r, expert_bias, active_mask)
    res = run_bass_kernel_spmd(nc, in_maps, list(range(N_CORES)), trace=trace, **kw)
    return _assemble(res.results, active_mask), res


def kernel(x, W_r, expert_bias, active_mask):
    return run(x, W_r, expert_bias, active_mask)[0]
